# revision 1
# baseline (speedup 1.0000x reference)
"""Trainium2 Bass kernel for LCAW-style supervised-contrastive loss.

Math (per anchor row i, all on-device except the final 4096-element sum):
    f      = features / ||features||                  (L2 normalize)
    sim    = f @ f.T                                  (cosine sims)
    A_i    = sum_{j: lbl_j==lbl_i} sim_ij (incl diag) (masked sum)
    n_i    = #{j: lbl_j==lbl_i} - 1                   (positives)
    d_i    = sim_ii (~1, computed as ||f_i||^2)
    L_i    = ln( sum_j exp(sim_ij) - exp(d_i) )       (logsumexp excl. diag;
             no stabilizer needed: sim <= 1 so exp(sim) <= e)
    loss_i = (n_i*L_i - (A_i - d_i)) / (n_i + 1e-5) * min(n_i, 1)
    loss   = sum_i loss_i / B
The min(n_i,1) gate makes singleton-label rows contribute exactly 0, matching
the reference's 0/(0+eps) = 0 without relying on fp cancellation.

Sharding: batch rows split across 8 cores (512 anchor rows each); full
features replicated to every core (2 MB) so no collectives are needed.
Host side only reformats inputs (tiling/transpose/broadcast of labels) and
sums the 4096 per-row results.
"""

import os
import sys

import numpy as np

for _p in ("/opt/trn_rl_repo", "/root/.axon_site/_ro/trn_rl_repo"):
    if os.path.isdir(_p) and _p not in sys.path:
        sys.path.insert(0, _p)

import concourse.bacc as bacc
import concourse.mybir as mybir
from concourse import tile
from concourse.bass_utils import run_bass_kernel_spmd

F32 = mybir.dt.float32
AF = mybir.ActivationFunctionType
ALU = mybir.AluOpType
AX = mybir.AxisListType

B, D = 4096, 128
N_CORES = 8
R = B // N_CORES      # 512 anchor rows per core
RT = R // 128         # 4 row tiles per core
NT = B // 128         # 32 column tiles of 128
NB = B // 512         # 8 matmul column blocks of 512

_NC_CACHE = {}


def build_nc():
    nc = bacc.Bacc(None, target_bir_lowering=False, debug=False)

    feat = nc.declare_dram_parameter("feat", [128, NT, 128], F32, isOutput=False)
    q = nc.declare_dram_parameter("q", [128, RT, 128], F32, isOutput=False)
    labq = nc.declare_dram_parameter("labq", [128, RT], F32, isOutput=False)
    labb = nc.declare_dram_parameter("labb", [128, B], F32, isOutput=False)
    ident = nc.declare_dram_parameter("ident", [128, 128], F32, isOutput=False)
    lossrows = nc.declare_dram_parameter("lossrows", [128, RT], F32, isOutput=True)

    with tile.TileContext(nc) as tc:
        with (
            tc.tile_pool(name="big", bufs=1) as big,
            tc.tile_pool(name="small", bufs=1) as small,
            tc.tile_pool(name="mask", bufs=2) as maskp,
            tc.tile_pool(name="work", bufs=4) as work,
            tc.tile_pool(name="psum_t", bufs=4, space="PSUM") as psum_t,
            tc.tile_pool(name="psum_s", bufs=4, space="PSUM") as psum_s,
        ):
            feats = big.tile([128, NT, 128], F32)   # row-major feature tiles
            fT = big.tile([128, B], F32)            # normalized, transposed (D on partitions)
            labb_sb = big.tile([128, B], F32)
            sqf = big.tile([128, NT, 128], F32)     # scratch for squares
            qrm = small.tile([128, RT, 128], F32)
            sqq = small.tile([128, RT, 128], F32)
            qT = small.tile([128, R], F32)
            labq_sb = small.tile([128, RT], F32)
            ident_sb = small.tile([128, 128], F32)
            ssq = small.tile([128, NT], F32)
            inv = small.tile([128, NT], F32)
            qssq = small.tile([128, RT], F32)
            qinv = small.tile([128, RT], F32)
            dcol = small.tile([128, RT], F32)
            ncol = small.tile([128, RT], F32)
            esum = small.tile([128, RT, NB], F32)
            apart = small.tile([128, RT, NB], F32)

            # ---- loads (all plain contiguous copies) ----
            nc.sync.dma_start(out=feats[:], in_=feat[:, :, :])
            nc.sync.dma_start(out=qrm[:], in_=q[:, :, :])
            nc.sync.dma_start(out=labq_sb[:], in_=labq[:, :])
            nc.sync.dma_start(out=labb_sb[:], in_=labb[:, :])
            nc.sync.dma_start(out=ident_sb[:], in_=ident[:, :])

            # ---- row norms: square on ACT, row-sum on DVE ----
            nc.scalar.activation(out=sqf[:], in_=feats[:], func=AF.Square)
            nc.vector.reduce_sum(ssq[:], sqf[:], axis=AX.X)
            nc.scalar.activation(out=sqq[:], in_=qrm[:], func=AF.Square)
            nc.vector.reduce_sum(qssq[:], sqq[:], axis=AX.X)
            nrm = small.tile([128, NT], F32)
            qnrm = small.tile([128, RT], F32)
            nc.scalar.activation(out=nrm[:], in_=ssq[:], func=AF.Sqrt)
            nc.scalar.activation(out=qnrm[:], in_=qssq[:], func=AF.Sqrt)
            nc.vector.reciprocal(out=inv[:], in_=nrm[:])
            nc.vector.reciprocal(out=qinv[:], in_=qnrm[:])

            # ---- normalize in place ----
            for j in range(NT):
                nc.vector.tensor_scalar_mul(feats[:, j, :], feats[:, j, :], inv[:, j : j + 1])
            for m in range(RT):
                nc.vector.tensor_scalar_mul(qrm[:, m, :], qrm[:, m, :], qinv[:, m : m + 1])

            # diagonal values d_i = ||f_i||^2 (post-normalization, ~1.0)
            nc.scalar.activation(out=sqq[:], in_=qrm[:], func=AF.Square)
            nc.vector.reduce_sum(dcol[:], sqq[:], axis=AX.X)

            # ---- transpose normalized features: fT[d, row] ----
            for j in range(NT):
                pt = psum_t.tile([128, 128], F32)
                nc.tensor.transpose(pt[:], feats[:, j, :], ident_sb[:])
                nc.scalar.copy(fT[:, j * 128 : (j + 1) * 128], pt[:])
            for m in range(RT):
                pt = psum_t.tile([128, 128], F32)
                nc.tensor.transpose(pt[:], qrm[:, m, :], ident_sb[:])
                nc.scalar.copy(qT[:, m * 128 : (m + 1) * 128], pt[:])

            # ---- main blocks: sim rows for this core's 512 anchors ----
            for m in range(RT):
                iseq = maskp.tile([128, B], F32, tag="iseq")
                nc.vector.tensor_scalar(
                    out=iseq[:], in0=labb_sb[:], scalar1=labq_sb[:, m : m + 1],
                    scalar2=None, op0=ALU.is_equal,
                )
                nc.vector.reduce_sum(ncol[:, m : m + 1], iseq[:], axis=AX.X)
                for n in range(NB):
                    ps = psum_s.tile([128, 512], F32)
                    nc.tensor.matmul(ps[:], lhsT=qT[:, m * 128 : (m + 1) * 128],
                                     rhs=fT[:, n * 512 : (n + 1) * 512],
                                     start=True, stop=True)
                    e_scr = work.tile([128, 512], F32, tag="e")
                    nc.scalar.activation(out=e_scr[:], in_=ps[:], func=AF.Exp)
                    nc.vector.reduce_sum(esum[:, m, n : n + 1], e_scr[:], axis=AX.X)
                    a_scr = work.tile([128, 512], F32, tag="a")
                    nc.vector.tensor_mul(a_scr[:], iseq[:, n * 512 : (n + 1) * 512], ps[:])
                    nc.vector.reduce_sum(apart[:, m, n : n + 1], a_scr[:], axis=AX.X)

            # ---- per-row scalars -> loss ----
            Esum = small.tile([128, RT], F32)
            Asum = small.tile([128, RT], F32)
            nc.vector.reduce_sum(Esum[:], esum[:], axis=AX.X)
            nc.vector.reduce_sum(Asum[:], apart[:], axis=AX.X)

            ecorr = small.tile([128, RT], F32)
            nc.scalar.activation(out=ecorr[:], in_=dcol[:], func=AF.Exp)
            S = small.tile([128, RT], F32)
            nc.vector.tensor_sub(S[:], Esum[:], ecorr[:])
            L = small.tile([128, RT], F32)
            nc.scalar.activation(out=L[:], in_=S[:], func=AF.Ln)

            n_nd = small.tile([128, RT], F32)
            nc.vector.tensor_scalar_add(n_nd[:], ncol[:], -1.0)
            ind = small.tile([128, RT], F32)
            nc.vector.tensor_scalar_min(ind[:], n_nd[:], 1.0)

            A_nd = small.tile([128, RT], F32)
            nc.vector.tensor_sub(A_nd[:], Asum[:], dcol[:])
            t1 = small.tile([128, RT], F32)
            nc.vector.tensor_mul(t1[:], n_nd[:], L[:])
            num = small.tile([128, RT], F32)
            nc.vector.tensor_sub(num[:], t1[:], A_nd[:])
            den = small.tile([128, RT], F32)
            nc.vector.tensor_scalar_add(den[:], n_nd[:], 1e-5)
            rec = small.tile([128, RT], F32)
            nc.vector.reciprocal(out=rec[:], in_=den[:])
            r1 = small.tile([128, RT], F32)
            nc.vector.tensor_mul(r1[:], num[:], rec[:])
            r2 = small.tile([128, RT], F32)
            nc.vector.tensor_mul(r2[:], r1[:], ind[:])

            nc.sync.dma_start(out=lossrows[:, :], in_=r2[:])

    nc.compile()
    return nc


def _get_nc():
    if "nc" not in _NC_CACHE:
        _NC_CACHE["nc"] = build_nc()
    return _NC_CACHE["nc"]


def make_in_maps(features, labels):
    feats = np.ascontiguousarray(np.asarray(features, dtype=np.float32))
    labs = np.ascontiguousarray(np.asarray(labels).astype(np.float32))
    # row-major 128-row tiles: [p, tile, d]
    feat_t = np.ascontiguousarray(feats.reshape(NT, 128, D).transpose(1, 0, 2))
    labb = np.ascontiguousarray(np.broadcast_to(labs[None, :], (128, B)))
    ident = np.eye(128, dtype=np.float32)
    in_maps = []
    for c in range(N_CORES):
        sl = slice(c * R, (c + 1) * R)
        q_t = np.ascontiguousarray(feats[sl].reshape(RT, 128, D).transpose(1, 0, 2))
        labq_t = np.ascontiguousarray(labs[sl].reshape(RT, 128).T)
        in_maps.append({
            "feat": feat_t,
            "q": q_t,
            "labq": labq_t,
            "labb": labb,
            "ident": ident,
        })
    return in_maps


def kernel(features, labels):
    nc = _get_nc()
    in_maps = make_in_maps(features, labels)
    res = run_bass_kernel_spmd(nc, in_maps, list(range(N_CORES))).results
    allrows = np.concatenate([r["lossrows"].reshape(-1) for r in res])
    return np.float32(allrows.sum(dtype=np.float32) / np.float32(B))



# revision 2
# speedup vs baseline: 12.9566x; 12.9566x over previous
"""Trainium2 Bass kernel for LCAW-style supervised-contrastive loss.

Math split:
  Device (O(B^2) work): each core owns 512 anchor rows. It receives ONLY its
  own L2-normalized feature shard, transposed, as fp16 [128(d), 512(rows)]
  (128 KB per core on the wire). An on-device HBM AllGather assembles the
  full [128, 4096] rhs; 32 fp16 matmuls produce sim rows in PSUM, and the
  scalar engine computes exp with a fused row-sum (accum_out) ->
      S_full[i] = sum_j exp(sim_ij)   (diagonal included).
  Host (O(B*D) numpy, overlapped with the device round trip): per-class
  feature sums give the masked positive-pair sums
      A_excl_i = sum_{j!=i, lbl_j==lbl_i} (f_i . f_j),
  counts n_i, and diag d_i = ||f_i||^2 ~ 1. Then
      L_i   = ln(S_full_i - e^{d_i})          (log-sum-exp excluding diag)
      pos_i = A_excl_i - n_i * L_i            (sum of logp over positives)
      loss  = -sum_i pos_i / (n_i + 1e-5) / B
  Rows with no positives contribute exactly 0 (A_excl is bitwise 0 there).

Execution: a jax.jit(shard_map(...)) runner wrapping the compiled Bass NEFF
is built once and cached; warm calls do a single dispatch round trip with
~530 KB H2D and 16 KB D2H. The host label math runs between the async
dispatch and the blocking fetch, so it adds nothing to the critical path.
Falls back to concourse.bass_utils.run_bass_kernel_spmd when the axon PJRT
redirect is not active (native NRT environments).
"""

import os
import sys

import numpy as np

for _p in ("/opt/trn_rl_repo", "/root/.axon_site/_ro/trn_rl_repo"):
    if os.path.isdir(_p) and _p not in sys.path:
        sys.path.insert(0, _p)

import concourse.bacc as bacc
import concourse.mybir as mybir
from concourse import tile

F32 = mybir.dt.float32
F16 = mybir.dt.float16
AF = mybir.ActivationFunctionType
ALU = mybir.AluOpType
AX = mybir.AxisListType

B, D = 4096, 128
N_CORES = 8
R = B // N_CORES      # 512 anchor rows per core
RT = R // 128         # 4 row tiles of 128 per core
NB = B // 512         # 8 column blocks of 512
NH = 2                # PSUM halves per row tile (4 banks = 2048 lanes each)

_CACHE = {}


def build_nc():
    nc = bacc.Bacc(None, target_bir_lowering=False, debug=False)

    # fsh[d, r] = normalized feature of global row (core*R + r), component d
    fsh = nc.declare_dram_parameter("fsh", [128, R], F16, isOutput=False)
    srows = nc.declare_dram_parameter("srows", [128, RT], F32, isOutput=True)
    inb = nc.dram_tensor("inb", [128, R], F16)
    gab = nc.dram_tensor("gab", [N_CORES, 128, R], F16)

    with tile.TileContext(nc) as tc:
        with (
            tc.tile_pool(name="sb", bufs=1) as sb,
            tc.tile_pool(name="work", bufs=2) as work,
            tc.tile_pool(name="ps4", bufs=2, space="PSUM") as psp4,
        ):
            lhs_sb = sb.tile([128, R], F16)
            rhs_sb = sb.tile([128, N_CORES, R], F16)
            esum = sb.tile([128, RT, NH], F32)
            srows_sb = sb.tile([128, RT], F32)

            nc.gpsimd.dma_start(out=inb[:, :], in_=fsh[:, :])
            nc.sync.dma_start(out=lhs_sb[:], in_=fsh[:, :])
            nc.gpsimd.collective_compute(
                "AllGather",
                ALU.bypass,
                replica_groups=[list(range(N_CORES))],
                ins=[inb.ap().opt()],
                outs=[gab.ap().opt()],
            )
            for c in range(N_CORES):
                eng = nc.sync if c % 2 == 0 else nc.scalar
                eng.dma_start(out=rhs_sb[:, c, :], in_=gab[c, :, :])

            for m in range(RT):
                for h in range(NH):
                    pt = psp4.tile([128, 2048], F32)
                    for n in range(4):
                        nc.tensor.matmul(
                            pt[:, n * 512 : (n + 1) * 512],
                            lhsT=lhs_sb[:, m * 128 : (m + 1) * 128],
                            rhs=rhs_sb[:, h * 4 + n, :],
                            start=True,
                            stop=True,
                        )
                    scr = work.tile([128, 2048], F16, tag="e")
                    nc.scalar.activation(
                        out=scr[:], in_=pt[:], func=AF.Exp,
                        accum_out=esum[:, m, h : h + 1],
                    )

            for m in range(RT):
                nc.vector.reduce_sum(srows_sb[:, m : m + 1], esum[:, m, :], axis=AX.X)
            nc.sync.dma_start(out=srows[:, :], in_=srows_sb[:])

    nc.compile()
    return nc


def _get_nc():
    if "nc" not in _CACHE:
        _CACHE["nc"] = build_nc()
    return _CACHE["nc"]


def _axon_active():
    try:
        from concourse.bass_utils import axon_active

        return axon_active()
    except Exception:
        return False


def _get_runner():
    """Build the jitted shard_map runner once; warm calls skip all tracing."""
    if "runner" in _CACHE:
        return _CACHE["runner"]

    import jax
    from jax.sharding import Mesh, PartitionSpec
    from jax.experimental.shard_map import shard_map
    from concourse.bass2jax import (
        _bass_exec_p,
        install_neuronx_cc_hook,
        partition_id_tensor,
    )

    nc = _get_nc()
    install_neuronx_cc_hook()
    assert nc.dbg_addr is None

    partition_name = nc.partition_id_tensor.name if nc.partition_id_tensor else None
    in_names, out_names, out_avals, zero_outs = [], [], [], []
    for alloc in nc.m.functions[0].allocations:
        if not isinstance(alloc, mybir.MemoryLocationSet):
            continue
        name = alloc.memorylocations[0].name
        if alloc.kind == "ExternalInput":
            if name != partition_name:
                in_names.append(name)
        elif alloc.kind == "ExternalOutput":
            out_names.append(name)
            shape = tuple(alloc.tensor_shape)
            dtype = mybir.dt.np(alloc.dtype)
            out_avals.append(jax.core.ShapedArray(shape, dtype))
            zero_outs.append(np.zeros(shape, dtype))
    n_params = len(in_names)
    n_outs = len(out_avals)
    in_names_all = in_names + out_names + ([partition_name] if partition_name else [])
    donate = tuple(range(n_params, n_params + n_outs))

    def _body(*args):
        operands = list(args)
        if partition_name is not None:
            operands.append(partition_id_tensor())
        outs = _bass_exec_p.bind(
            *operands,
            out_avals=tuple(out_avals),
            in_names=tuple(in_names_all),
            out_names=tuple(out_names),
            lowering_input_output_aliases=(),
            sim_require_finite=True,
            sim_require_nnan=True,
            nc=nc,
        )
        return tuple(outs)

    devices = jax.devices()[:N_CORES]
    assert len(devices) == N_CORES
    mesh = Mesh(np.asarray(devices), ("core",))
    in_specs = (PartitionSpec("core"),) * (n_params + n_outs)
    out_specs = (PartitionSpec("core"),) * len(out_names)
    sharded = jax.jit(
        shard_map(_body, mesh=mesh, in_specs=in_specs, out_specs=out_specs,
                  check_rep=False),
        donate_argnums=donate,
        keep_unused=True,
    )
    runner = {"fn": sharded, "zero_outs": zero_outs}
    _CACHE["runner"] = runner
    return runner


def _make_fsh(f32norm):
    """fp16 global input: core c's block rows [c*128,(c+1)*128) hold its
    transposed shard fT_c[d, r] = f[c*R + r, d]."""
    fsh = f32norm.reshape(N_CORES, R, D).transpose(0, 2, 1).astype(np.float16)
    return np.ascontiguousarray(fsh).reshape(N_CORES * 128, R)


def _unpack_S(out):
    # element [c*128+p, m] = S for anchor row c*R + m*128 + p
    return out.reshape(N_CORES, 128, RT).transpose(0, 2, 1).reshape(B)


def _dispatch_axon(fsh):
    r = _get_runner()
    zeros = [
        np.zeros((N_CORES * z.shape[0], *z.shape[1:]), z.dtype)
        for z in r["zero_outs"]
    ]
    outs = r["fn"](fsh, *zeros)  # async under jax dispatch

    def fetch():
        return _unpack_S(np.asarray(outs[0]))

    return fetch


def _dispatch_native(fsh):
    from concourse.bass_utils import run_bass_kernel_spmd

    nc = _get_nc()
    fsh3 = fsh.reshape(N_CORES, 128, R)
    in_maps = [{"fsh": fsh3[c]} for c in range(N_CORES)]

    def fetch():
        res = run_bass_kernel_spmd(nc, in_maps, list(range(N_CORES))).results
        out = np.concatenate([r["srows"] for r in res], axis=0)
        return _unpack_S(out)

    return fetch


def kernel(features, labels):
    feats = np.ascontiguousarray(np.asarray(features, dtype=np.float32))
    labs = np.asarray(labels).reshape(-1)

    nrm = np.sqrt(np.einsum("ij,ij->i", feats, feats))
    f = feats / np.maximum(nrm, 1e-12)[:, None]

    fetch = (_dispatch_axon if _axon_active() else _dispatch_native)(_make_fsh(f))

    # ---- host label math, overlapped with the device round trip ----
    d = np.einsum("ij,ij->i", f, f)  # ~1.0, matches device diag closely
    order = np.argsort(labs, kind="stable")
    slab = labs[order]
    newcls = np.r_[True, slab[1:] != slab[:-1]]
    starts = np.flatnonzero(newcls)
    csums = np.add.reduceat(f[order], starts, axis=0)  # per-class sums
    cnt = np.diff(np.r_[starts, len(slab)])
    cidx = np.empty(B, np.int64)
    cidx[order] = np.cumsum(newcls) - 1
    g = csums[cidx]                       # per-row same-class feature sum
    n_nd = cnt[cidx].astype(np.float64) - 1.0
    A_excl = (np.einsum("ij,ij->i", f, g) - d).astype(np.float64)

    S_full = fetch().astype(np.float64)

    S = S_full - np.exp(d.astype(np.float64))
    L = np.log(S)
    pos = A_excl - n_nd * L
    li = -pos / (n_nd + 1e-5)
    li[n_nd < 0.5] = 0.0
    return np.float32(li.sum() / B)
